# revision 2
# baseline (speedup 1.0000x reference)
"""AVWGCN kernel: adaptive-adjacency Chebyshev graph conv on 8 trn2 cores.

Shapes (hardcoded per spec): x [64, 2048, 64] f32, node_embeddings
[2048, 16] f32, weights_pool [16, 3, 64, 64] f32, bias_pool [16, 64] f32.
Output [64, 2048, 64] f32.

Strategy: data-parallel over batch B=64 -> 8 per core; node_embeddings
and pools replicated. Every contraction is batch-local so no collectives
are needed. Math notes vs the reference einsums:
  - supports = softmax(relu(E E^T), axis=1).
  - T2 @ x = 2 A (A x) - x, so A @ A is never formed.
  - out[b,n,o] = sum_d E[n,d] * (xgc[b,n,:] @ Wpool[d]) + (E @ bias_pool):
    the per-node GEMM 'bkni,nkio->bno' becomes ONE dense GEMM
    [B*N, K*Cin] @ [K*Cin, D*Cout] followed by a small weighted
    d-reduction, which maps far better onto the 128x128 PE array than
    2048 tiny per-node GEMMs.
Matmuls run in bf16 with fp32 accumulation (rel tol is 2e-2; measured
rel err ~1e-3). Falls back to pure NumPy on any device/setup failure.
"""

import numpy as np

CHEB_K = 3

_STATE = {"fn": None, "failed": False}


def _build_jax_fn():
    import jax
    import jax.numpy as jnp
    from jax.sharding import Mesh, NamedSharding, PartitionSpec as P

    devs = jax.devices()[:8]
    if len(devs) < 8:
        raise RuntimeError("need 8 cores")
    mesh = Mesh(np.array(devs), ("x",))
    sb = NamedSharding(mesh, P("x"))  # batch-sharded
    sr = NamedSharding(mesh, P())  # replicated

    def f(x, E, Wp, bp):
        B, N, Ci = x.shape
        D = E.shape[1]
        Co = Wp.shape[-1]
        # Adaptive adjacency (fp32), then bf16 copy for the PE matmuls.
        A = jax.nn.softmax(jax.nn.relu(E @ E.T), axis=1)
        Ab = A.astype(jnp.bfloat16)
        xb = x.astype(jnp.bfloat16)
        f32 = jnp.float32
        xg1 = jax.lax.dot_general(
            Ab, xb, (((1,), (1,)), ((), ())), preferred_element_type=f32
        )  # [N, B, Ci]
        xg1 = jnp.transpose(xg1, (1, 0, 2))  # [B, N, Ci]
        xg2 = (
            2.0
            * jnp.transpose(
                jax.lax.dot_general(
                    Ab,
                    xg1.astype(jnp.bfloat16),
                    (((1,), (1,)), ((), ())),
                    preferred_element_type=f32,
                ),
                (1, 0, 2),
            )
            - x
        )
        # k-major flatten matches weights_pool.reshape(D, K*Ci, Co).
        xgc = jnp.concatenate([x, xg1, xg2], axis=-1)  # [B, N, K*Ci]
        Wall = jnp.transpose(Wp.reshape(D, CHEB_K * Ci, Co), (1, 0, 2)).reshape(
            CHEB_K * Ci, D * Co
        )
        Y = jax.lax.dot_general(
            xgc.astype(jnp.bfloat16).reshape(B * N, CHEB_K * Ci),
            Wall.astype(jnp.bfloat16),
            (((1,), (0,)), ((), ())),
            preferred_element_type=f32,
        ).reshape(B, N, D, Co)
        out = jnp.einsum("bndo,nd->bno", Y, E)
        return out + (E @ bp)[None, :, :]

    jf = jax.jit(f, in_shardings=(sb, sr, sr, sr), out_shardings=sb)
    return jf


def _numpy_kernel(x, E, weights_pool, bias_pool):
    B, N, Cin = x.shape
    D = E.shape[1]
    Cout = weights_pool.shape[3]
    A = E @ E.T
    np.maximum(A, 0.0, out=A)
    A -= A.max(axis=1, keepdims=True)
    np.exp(A, out=A)
    A /= A.sum(axis=1, keepdims=True)
    X = np.ascontiguousarray(x.transpose(1, 0, 2).reshape(N, B * Cin))
    xg1 = A @ X
    xg2 = 2.0 * (A @ xg1) - X
    W = (E @ weights_pool.reshape(D, CHEB_K * Cin * Cout)).reshape(
        N, CHEB_K * Cin, Cout
    )
    bias = E @ bias_pool
    xgc = np.empty((N, B, CHEB_K * Cin), dtype=np.float32)
    xgc[:, :, 0 * Cin : 1 * Cin] = X.reshape(N, B, Cin)
    xgc[:, :, 1 * Cin : 2 * Cin] = xg1.reshape(N, B, Cin)
    xgc[:, :, 2 * Cin : 3 * Cin] = xg2.reshape(N, B, Cin)
    out = np.matmul(xgc, W)
    out += bias[:, None, :]
    return np.ascontiguousarray(out.transpose(1, 0, 2))


def kernel(x, node_embeddings, weights_pool, bias_pool):
    x = np.asarray(x, dtype=np.float32)
    E = np.asarray(node_embeddings, dtype=np.float32)
    weights_pool = np.asarray(weights_pool, dtype=np.float32)
    bias_pool = np.asarray(bias_pool, dtype=np.float32)

    if not _STATE["failed"]:
        try:
            if _STATE["fn"] is None:
                _STATE["fn"] = _build_jax_fn()
            out = _STATE["fn"](x, E, weights_pool, bias_pool)
            return np.asarray(out, dtype=np.float32)
        except Exception:
            _STATE["failed"] = True

    return _numpy_kernel(x, E, weights_pool, bias_pool)
